# revision 1
# baseline (speedup 1.0000x reference)
"""DimeNet++ interaction block on 8 Trainium2 NeuronCores (Bass/Tile).

Strategy (v2):
- Sort triplets by idx_ji, shard by 25k-edge ranges per core (no all-reduce:
  each core owns its output edge slice).
- Within a core, triplets grouped into 128-edge windows of idx_ji with fixed
  per-window slot capacity (cap_tiles*128) so one SPMD program serves all
  cores. Windows processed in PAIRS: window A accumulates in PSUM partitions
  0-63, window B in 64-127 (tensor-engine column tiling).
- sbf basis is folded on host: se = (sbf@W_sbf1)@W_sbf2 shipped as [T,64] f16.
- x_kj down-projection table [E,64] f16 computed E-sharded, AllGathered.
- Gathers batched: ONE indirect DMA per window pair (2*cap*128 rows).
- segment_sum via one-hot matmuls accumulated directly in PSUM (start/stop).
- All on-chip compute f16 (PSUM accumulation f32); output written transposed
  [H, E] and transposed back on host.
"""
import sys
import os as _os2
import numpy as np

sys.path.insert(0, "/opt/trn_rl_repo")

N_CORES = 8
WIN_E = 128          # edges per segment-sum window (one PSUM column block)
TILE_T = 128         # triplet slots per matmul tile


def _apply_tile_patches():
    """walrus in this container allows only 1 sync wait per instruction; split
    the TileContext tail drain into a chain of single-wait NOPs. Also register
    the NTFF profile hook so trace=True works (used by test harness only)."""
    import types
    import concourse.tile as tile
    from concourse.vector_clock import ScopedClock

    def _drain_and_barrier_split(self, tick_clock, wait_clock):
        gc = tick_clock.global_clock
        procs = [i for i in range(len(gc)) if gc[i] > 0]
        chunks = [procs[i : i + 1] for i in range(len(procs))]
        for ch in chunks[:-1] if chunks else []:
            nop = self.nc.sync.nop()
            pc = ScopedClock()
            for p in ch:
                pc.require_at_least(None, p, gc[p])
            wait_clock.add_sem_waits(nop.ins, pc)
        drain_inst = self.nc.sync.drain()
        pc = ScopedClock()
        for p in chunks[-1] if chunks else []:
            pc.require_at_least(None, p, gc[p])
        wait_clock.add_sem_waits(drain_inst.ins, pc)
        self.nc.all_engine_barrier()
        assert self.sems is not None
        popped = self.nc._tile_sem_poison_stack.pop()
        assert popped is self._sem_poison
        self.nc.clear_and_free_semaphores(list(self.sems.allocated().values()))
        self.nc.all_engine_barrier()

    tile.TileContext._drain_and_barrier = _drain_and_barrier_split

    if "antenv.axon_hooks" not in sys.modules:
        mod = types.ModuleType("antenv.axon_hooks")
        _state = {"hook": None}
        mod.set_axon_ntff_profile_hook = lambda h: _state.__setitem__("hook", h)
        mod.get_axon_ntff_profile_hook = lambda: _state["hook"]
        sys.modules["antenv.axon_hooks"] = mod
        import antenv

        antenv.axon_hooks = mod
        try:
            from trn_agent_boot.trn_boot import _ntff_profile_via_ctypes

            hook = _ntff_profile_via_ctypes("/opt/axon/libaxon_pjrt.so")
            if hook is not None:
                mod.set_axon_ntff_profile_hook(hook)
        except Exception:
            pass


N_CHUNK = 7
G_PAIRS = 4


def _build_program(E, H, NR, e_core, e_pad, n_pairs, cap_tiles,
                   NB_BEFORE, NB_AFTER):
    import concourse.bass as bass
    import concourse.bacc as bacc
    import concourse.tile as tile
    from concourse import mybir
    from contextlib import ExitStack

    f16, f32 = mybir.dt.float16, mybir.dt.float32
    AF = mybir.ActivationFunctionType
    if _os2.environ.get("KRELU"):
        AF = type("AFX", (), {"Silu": mybir.ActivationFunctionType.Relu,
                              "Copy": mybir.ActivationFunctionType.Copy})
    OP = mybir.AluOpType
    PHASES = _os2.environ.get("PH", "ABC")
    BLV = int(_os2.environ.get("BLV", "3"))

    NB512 = e_pad // 512
    e_tab = N_CORES * e_pad
    n_res = NB_BEFORE * 2 + 1 + NB_AFTER * 2
    cap_c = cap_tiles
    n_groups = n_pairs // G_PAIRS
    assert n_pairs % G_PAIRS == 0
    chunk_rows = e_tab // N_CHUNK
    assert e_tab % N_CHUNK == 0 and chunk_rows <= 32768
    bp = N_CHUNK * 2 * cap_c            # gather blocks per pair
    bg = bp * G_PAIRS                   # gather blocks per group
    n_idx = G_PAIRS * 2 * cap_c * 128   # indices per gather call
    nidx16 = n_idx // 16

    nc = bacc.Bacc("TRN2", target_bir_lowering=False, debug=False,
                   num_devices=N_CORES)

    def din(name, shape, dt):
        return nc.dram_tensor(name, shape, dt, kind="ExternalInput")

    xT_in = din("xT", [H, e_pad], f16)
    rbfT_in = din("rbfT", [NR, e_pad], f16)
    se_in = din("se", [n_groups, 128, bg * 64], f16)
    gidx_in = din("gidx", [n_groups, N_CHUNK, 128, nidx16], mybir.dt.int16)
    jil_in = din("jil", [n_pairs, 128, bp, 1], f16)
    iota_in = din("iota", [128, bp, WIN_E], f16)
    w_ji_in = din("w_ji", [H, H], f16)
    b_ji_in = din("b_ji", [H, 1], f32)
    w_kj_in = din("w_kj", [H, H], f16)
    b_kj_in = din("b_kj", [H, 1], f32)
    wc_rbf_in = din("wc_rbf", [NR, H], f16)
    w_down_in = din("w_down", [H, 64], f16)
    w_up2_in = din("w_up2", [128, H], f16)
    w_res_in = din("w_res", [H, n_res * H], f16)
    b_res_in = din("b_res", [H, n_res], f32)

    out_ext = nc.dram_tensor("out", [H, e_pad], f16, kind="ExternalOutput")

    with tile.TileContext(nc) as tc, ExitStack() as ctx:
        const = ctx.enter_context(tc.tile_pool(name="const", bufs=1))
        persist = ctx.enter_context(tc.tile_pool(name="persist", bufs=1))
        dram = ctx.enter_context(tc.tile_pool(name="dram", bufs=1, space="DRAM"))
        tab_slice = dram.tile([e_pad, 128], f16, tag="tab_slice")
        tab_full = dram.tile([e_tab, 128], f16, tag="tab_full")
        agg_hi_d = dram.tile([64, n_pairs * WIN_E], f16, tag="agg_hi")

        def load_const(ap_in, shape, dt, tag):
            t = const.tile(shape, dt, tag=tag)
            nc.sync.dma_start(t[:], ap_in[:])
            return t

        iota = load_const(iota_in, [128, bp, WIN_E], f16, "c_iota")
        w_ji = load_const(w_ji_in, [H, H], f16, "c_wji")
        b_ji = load_const(b_ji_in, [H, 1], f32, "c_bji")
        w_kj = load_const(w_kj_in, [H, H], f16, "c_wkj")
        b_kj = load_const(b_kj_in, [H, 1], f32, "c_bkj")
        wc_rbf = load_const(wc_rbf_in, [NR, H], f16, "c_wcrbf")
        w_down = load_const(w_down_in, [H, 64], f16, "c_wdown")
        w_up2 = load_const(w_up2_in, [128, H], f16, "c_wup2")
        w_res = load_const(w_res_in, [H, n_res * H], f16, "c_wres")
        b_res = load_const(b_res_in, [H, n_res], f32, "c_bres")

        agg = persist.tile([128, n_pairs * WIN_E], f16)

        # ---------------- phase A: x_kj down-projection table ------------
        with (
            tc.tile_pool(name="a_sb", bufs=3) as a_sb,
            tc.tile_pool(name="a_ps", bufs=2, space="PSUM") as a_ps,
        ):
            for blk in range(NB512 if "A" in PHASES else 0):
                sl = slice(blk * 512, (blk + 1) * 512)
                xT = a_sb.tile([H, 512], f16, tag="xT")
                nc.sync.dma_start(xT[:], xT_in[:, sl])
                rbfT = a_sb.tile([NR, 512], f16, tag="rbfT")
                nc.sync.dma_start(rbfT[:], rbfT_in[:, sl])

                ps_kj = a_ps.tile([H, 512], f32, tag="psA")
                nc.tensor.matmul(out=ps_kj[:], lhsT=w_kj[:], rhs=xT[:],
                                 start=True, stop=True)
                t1 = a_sb.tile([H, 512], f16, tag="t1")
                nc.scalar.activation(t1[:], ps_kj[:], AF.Silu, bias=b_kj[:])

                ps_rbf = a_ps.tile([H, 512], f32, tag="psA")
                nc.tensor.matmul(out=ps_rbf[:], lhsT=wc_rbf[:], rhs=rbfT[:],
                                 start=True, stop=True)
                t2 = a_sb.tile([H, 512], f16, tag="t2")
                nc.vector.tensor_tensor(out=t2[:], in0=t1[:], in1=ps_rbf[:],
                                        op=OP.mult)
                td = a_sb.tile([128, 4, 64], f16, tag="td")
                for j in range(4):
                    ps_d = a_ps.tile([128, 64], f32, tag="psD")
                    nc.tensor.matmul(out=ps_d[:],
                                     lhsT=t2[:, j * 128:(j + 1) * 128],
                                     rhs=w_down[:], start=True, stop=True)
                    nc.scalar.activation(td[:, j, :], ps_d[:], AF.Silu)
                nc.sync.dma_start(
                    tab_slice[blk * 512:(blk + 1) * 512, 0:64].rearrange(
                        "(b p) d -> p b d", p=128), td[:])

        # ---------------- AllGather the table ----------------------------
        if "G" in PHASES or "B" in PHASES:
            nc.gpsimd.collective_compute(
                "AllGather", OP.bypass,
                replica_groups=[list(range(N_CORES))],
                ins=[tab_slice.opt()],
                outs=[tab_full.opt()],
            )

        # ---------------- phase B: triplets ------------------------------
        with (
            tc.tile_pool(name="b_io", bufs=2) as b_io,
            tc.tile_pool(name="b_gat", bufs=2) as b_gat,
            tc.tile_pool(name="b_oh", bufs=3) as b_oh,
            tc.tile_pool(name="b_ps", bufs=2, space="PSUM") as b_ps,
        ):
            for g in range(n_groups if "B" in PHASES else 0):
                se_t = b_io.tile([128, bg, 64], f16, tag="se")
                nc.sync.dma_start(
                    se_t[:].rearrange("p b d -> p (b d)"), se_in[g])
                gi_t = b_io.tile([128, N_CHUNK, nidx16], mybir.dt.int16,
                                 tag="gi")
                nc.sync.dma_start(
                    gi_t[:], gidx_in[g].rearrange("c p s -> p c s"))

                gat = b_gat.tile([128, bg, 128], f16, tag="gat")
                if BLV >= 1:
                    for c in range(N_CHUNK):
                        nc.gpsimd.dma_gather(
                            gat[:, c * (bg // N_CHUNK):(c + 1) * (bg // N_CHUNK), :],
                            tab_full[c * chunk_rows:(c + 1) * chunk_rows, :],
                            gi_t[:, c, :],
                            n_idx, n_idx, 128, single_packet=False)

                m = b_gat.tile([128, bg, 64], f16, tag="m")
                if BLV >= 3:
                    nc.vector.tensor_tensor(
                        out=m[:], in0=gat[:, :, 0:64],
                        in1=se_t[:], op=OP.mult)

                for pig in range(G_PAIRS):
                    p = g * G_PAIRS + pig
                    jl_t = b_oh.tile([128, bp, 1], f16, tag="jl")
                    nc.sync.dma_start(jl_t[:], jil_in[p])
                    oh = b_oh.tile([128, bp, WIN_E], f16, tag="oh")
                    if BLV >= 2:
                        nc.vector.tensor_tensor(
                            out=oh[:], in0=iota[:],
                            in1=jl_t[:].broadcast_to([128, bp, WIN_E]),
                            op=OP.is_equal)
                    if BLV >= 3:
                        ps = b_ps.tile([128, WIN_E], f32, tag="ps")
                        for half in range(2):
                            nmm = 0
                            for c in range(N_CHUNK):
                                for t in range(cap_c):
                                    mb_ = (c * G_PAIRS + pig) * 2 * cap_c \
                                        + half * cap_c + t
                                    ob_ = c * 2 * cap_c + half * cap_c + t
                                    nc.tensor.matmul(
                                        out=ps[64 * half:64 * half + 64, :],
                                        lhsT=m[:, mb_, :],
                                        rhs=oh[:, ob_, :],
                                        start=(nmm == 0),
                                        stop=(nmm == N_CHUNK * cap_c - 1))
                                    nmm += 1
                        nc.scalar.activation(
                            agg[:, p * WIN_E:(p + 1) * WIN_E], ps[:], AF.Copy)
                if g == 0 and _os2.environ.get("KDBG"):
                    nc.gpsimd.dma_start(
                        out_ext[:, 0:bg * 64],
                        gat[:, :, 0:64].rearrange("p c d -> p (c d)"))
                    nc.gpsimd.dma_start(
                        out_ext[:, bg * 64:2 * bg * 64],
                        m[:].rearrange("p c d -> p (c d)"))

        # move window-B halves (PSUM partitions 64-127) out through DRAM so
        # phase C matmuls only read base-0 operands (base-64 reads fault HW)
        if "B" in PHASES and "C" in PHASES:
            nc.sync.dma_start(agg_hi_d[:], agg[64:128, :])
        if "B" in PHASES and _os2.environ.get("KDBG"):
            nc.gpsimd.dma_start(
                out_ext[:, 2 * bg * 64:e_pad],
                agg[:, 0:e_pad - 2 * bg * 64])

        # ---------------- phase C: edge output ---------------------------
        with (
            tc.tile_pool(name="c_sb", bufs=3) as c_sb,
            tc.tile_pool(name="c_ps", bufs=2, space="PSUM") as c_ps,
        ):
            for blk in range(NB512 if "C" in PHASES else 0):
                sl = slice(blk * 512, (blk + 1) * 512)
                xT = c_sb.tile([H, 512], f16, tag="xT2")
                nc.sync.dma_start(xT[:], xT_in[:, sl])
                ahi = c_sb.tile([64, 256], f16, tag="ahi")
                nc.sync.dma_start(
                    ahi[:], agg_hi_d[:, 2 * blk * WIN_E:(2 * blk + 2) * WIN_E])

                ps_u = c_ps.tile([H, 512], f32, tag="psU")
                for q in range(4):
                    pr = 2 * blk + q // 2
                    if q % 2 == 0:
                        rhs = agg[0:64, pr * WIN_E:(pr + 1) * WIN_E]
                    else:
                        rhs = ahi[:, (q // 2) * WIN_E:(q // 2 + 1) * WIN_E]
                    nc.tensor.matmul(
                        out=ps_u[:, q * 128:(q + 1) * 128],
                        lhsT=w_up2[0:64, :],
                        rhs=rhs,
                        start=True, stop=True)
                xkju = c_sb.tile([H, 512], f16, tag="xkju")
                nc.scalar.activation(xkju[:], ps_u[:], AF.Silu)

                ps_j = c_ps.tile([H, 512], f32, tag="psU")
                nc.tensor.matmul(out=ps_j[:], lhsT=w_ji[:], rhs=xT[:],
                                 start=True, stop=True)
                xji = c_sb.tile([H, 512], f16, tag="xji")
                nc.scalar.activation(xji[:], ps_j[:], AF.Silu, bias=b_ji[:])

                h = c_sb.tile([H, 512], f16, tag="h")
                nc.vector.tensor_tensor(out=h[:], in0=xji[:], in1=xkju[:],
                                        op=OP.add)

                def res_layer(h_in, li):
                    ps_a = c_ps.tile([H, 512], f32, tag="psU")
                    nc.tensor.matmul(out=ps_a[:],
                                     lhsT=w_res[:, li * H:(li + 1) * H],
                                     rhs=h_in[:], start=True, stop=True)
                    inner = c_sb.tile([H, 512], f16, tag="inner")
                    nc.scalar.activation(inner[:], ps_a[:], AF.Silu,
                                         bias=b_res[:, li:li + 1])
                    ps_b = c_ps.tile([H, 512], f32, tag="psU")
                    nc.tensor.matmul(out=ps_b[:],
                                     lhsT=w_res[:, (li + 1) * H:(li + 2) * H],
                                     rhs=inner[:], start=True, stop=True)
                    s = c_sb.tile([H, 512], f16, tag="s")
                    nc.scalar.activation(s[:], ps_b[:], AF.Silu,
                                         bias=b_res[:, li + 1:li + 2])
                    h_out = c_sb.tile([H, 512], f16, tag="h")
                    nc.vector.tensor_tensor(out=h_out[:], in0=h_in[:],
                                            in1=s[:], op=OP.add)
                    return h_out

                li = 0
                for _ in range(NB_BEFORE):
                    h = res_layer(h, li)
                    li += 2
                ps_l = c_ps.tile([H, 512], f32, tag="psU")
                nc.tensor.matmul(out=ps_l[:],
                                 lhsT=w_res[:, li * H:(li + 1) * H],
                                 rhs=h[:], start=True, stop=True)
                sl_t = c_sb.tile([H, 512], f16, tag="s")
                nc.scalar.activation(sl_t[:], ps_l[:], AF.Silu,
                                     bias=b_res[:, li:li + 1])
                li += 1
                h = c_sb.tile([H, 512], f16, tag="h")
                nc.vector.tensor_tensor(out=h[:], in0=sl_t[:], in1=xT[:],
                                        op=OP.add)
                for r in range(NB_AFTER):
                    h = res_layer(h, li)
                    li += 2
                nc.sync.dma_start(out_ext[:, sl], h[:])

    nc.compile()
    return nc


def kernel(**inputs):
    _apply_tile_patches()
    from concourse.bass_utils import run_bass_kernel_spmd

    x = np.asarray(inputs["x"], np.float32)
    rbf = np.asarray(inputs["rbf"], np.float32)
    sbf = np.asarray(inputs["sbf"], np.float32)
    idx_kj = np.asarray(inputs["idx_kj"]).astype(np.int64)
    idx_ji = np.asarray(inputs["idx_ji"]).astype(np.int64)

    E, H = x.shape
    T, NS_NR = sbf.shape
    NR = rbf.shape[1]
    W_res_before = np.asarray(inputs["W_res_before"], np.float32)
    W_res_after = np.asarray(inputs["W_res_after"], np.float32)
    b_res_before = np.asarray(inputs["b_res_before"], np.float32)
    b_res_after = np.asarray(inputs["b_res_after"], np.float32)
    NB_BEFORE = W_res_before.shape[0]
    NB_AFTER = W_res_after.shape[0]

    assert E % N_CORES == 0
    e_core = E // N_CORES
    e_pad = -(-e_core // 512) * 512
    n_win = e_pad // WIN_E
    assert n_win % 2 == 0
    n_pairs_real = n_win // 2
    n_pairs = -(-n_pairs_real // G_PAIRS) * G_PAIRS   # padded to group size
    n_groups = n_pairs // G_PAIRS
    e_tab = N_CORES * e_pad
    chunk_rows = e_tab // N_CHUNK
    assert e_tab % N_CHUNK == 0 and chunk_rows <= 32768

    # ---------------- host-side preprocessing -------------------------
    # fold sbf basis: se = (sbf @ W_sbf1) @ W_sbf2  -> [T, 64] f16
    se_full = ((sbf @ np.asarray(inputs["W_sbf1"], np.float32))
               @ np.asarray(inputs["W_sbf2"], np.float32)).astype(np.float16)
    I_DIM = se_full.shape[1]
    assert I_DIM == 64

    order = np.argsort(idx_ji, kind="stable")
    ji_sorted = idx_ji[order]
    bounds = np.searchsorted(ji_sorted, np.arange(N_CORES + 1) * e_core)
    kj_row_all = (idx_kj // e_core) * e_pad + idx_kj % e_core
    kj_chunk_all = (kj_row_all // chunk_rows).astype(np.int16)
    kj_loc_all = (kj_row_all % chunk_rows).astype(np.int16)

    percore = []
    max_cell = 0
    n_cells = n_win * N_CHUNK
    for k in range(N_CORES):
        oj = order[bounds[k]:bounds[k + 1]]
        ji_l = ji_sorted[bounds[k]:bounds[k + 1]] - k * e_core
        w = ji_l // WIN_E
        cell = w * N_CHUNK + kj_chunk_all[oj]
        counts = np.bincount(cell, minlength=n_cells)
        max_cell = max(max_cell, int(counts.max()) if len(counts) else 0)
        percore.append((oj, ji_l, cell, counts))

    cap_c = max(1, -(-max_cell // TILE_T))
    cell_slots = cap_c * TILE_T
    bp = N_CHUNK * 2 * cap_c
    bg = bp * G_PAIRS
    n_idx = G_PAIRS * 2 * cap_c * 128
    nidx16 = n_idx // 16
    n_slots = n_pairs * 2 * N_CHUNK * cell_slots

    in_maps = []
    wc_rbf = (np.asarray(inputs["W_rbf1"], np.float32)
              @ np.asarray(inputs["W_rbf2"], np.float32)).astype(np.float16)
    w_up2 = np.concatenate([np.asarray(inputs["W_up"], np.float32)] * 2,
                           axis=0).astype(np.float16)
    n_res = NB_BEFORE * 2 + 1 + NB_AFTER * 2
    w_res = np.concatenate([
        W_res_before.reshape(-1, H, H),
        np.asarray(inputs["W_lin"], np.float32)[None],
        W_res_after.reshape(-1, H, H)])
    w_res = np.ascontiguousarray(
        w_res.transpose(1, 0, 2).reshape(H, -1)).astype(np.float16)
    b_res = np.concatenate([
        b_res_before.reshape(-1, H),
        np.asarray(inputs["b_lin"], np.float32)[None],
        b_res_after.reshape(-1, H)])
    b_res = np.ascontiguousarray(b_res.T)  # [H, n_res] f32
    # iota[p, b, e] = e
    iota = np.broadcast_to(
        np.arange(WIN_E, dtype=np.float16)[None, None, :],
        (128, bp, WIN_E)).copy()

    for k in range(N_CORES):
        oj, ji_l, cell, counts = percore[k]
        # order triplets by (cell, table-row) for gather locality
        ord2 = np.lexsort((kj_loc_all[oj], cell))
        oj = oj[ord2]
        ji_l = ji_l[ord2]
        cell = cell[ord2]
        starts = np.zeros(n_cells, np.int64)
        np.cumsum(counts[:-1], out=starts[1:])
        rank = np.arange(len(cell)) - starts[cell]
        # storage block id: [g, chunk, pig, w, t] major -> part minor
        w_full = cell // N_CHUNK
        c_ch = cell % N_CHUNK
        pr = w_full >> 1
        w_in = w_full & 1
        g_id = pr // G_PAIRS
        pig = pr % G_PAIRS
        t_id = rank // TILE_T
        part = rank % TILE_T
        blk = (((g_id * N_CHUNK + c_ch) * G_PAIRS + pig) * 2 * cap_c
               + w_in * cap_c + t_id)
        slots = blk * TILE_T + part

        se_slots = np.zeros((n_slots, 64), np.float16)
        se_slots[slots] = se_full[oj]
        gidx_flat = np.zeros(n_slots, np.int16)
        gidx_flat[slots] = kj_loc_all[oj]
        jil_flat = np.full(n_slots, -1.0, np.float16)
        jil_flat[slots] = (ji_l % WIN_E).astype(np.float16)

        # se: [g, c, pig, w, t, 128, 64] -> [g, 128, (c pig w t), 64]
        se_r = se_slots.reshape(n_groups, N_CHUNK, G_PAIRS, 2, cap_c, 128, 64)
        se_arr = np.ascontiguousarray(
            se_r.transpose(0, 5, 1, 2, 3, 4, 6).reshape(n_groups, 128,
                                                        bg * 64))
        # gidx: per (g, c): flat (pig, w, t, part) -> [16, nidx16] wrap,
        # replicated to all 8 Q7 partition groups (128 partitions)
        gi_r = gidx_flat.reshape(n_groups, N_CHUNK, n_idx)
        gi16 = gi_r.reshape(n_groups, N_CHUNK, nidx16, 16).transpose(0, 1, 3, 2)
        gi_arr = np.ascontiguousarray(
            np.tile(gi16, (1, 1, 8, 1)))  # [g, c, 128, nidx16]
        # jil: [g, pig, 128, c, w, t] -> [n_pairs, 128, bp, 1]
        jl_r = jil_flat.reshape(n_groups, N_CHUNK, G_PAIRS, 2, cap_c, 128)
        jl_arr = np.ascontiguousarray(
            jl_r.transpose(0, 2, 5, 1, 3, 4).reshape(n_pairs, 128, bp, 1))

        xT = np.zeros((H, e_pad), np.float16)
        xT[:, :e_core] = x[k * e_core:(k + 1) * e_core].T
        rbfT = np.zeros((NR, e_pad), np.float16)
        rbfT[:, :e_core] = rbf[k * e_core:(k + 1) * e_core].T

        in_maps.append({
            "xT": xT, "rbfT": rbfT, "se": se_arr, "gidx": gi_arr,
            "jil": jl_arr, "iota": iota,
            "w_ji": np.asarray(inputs["W_ji"], np.float32).astype(np.float16),
            "b_ji": np.asarray(inputs["b_ji"], np.float32)[:, None],
            "w_kj": np.asarray(inputs["W_kj"], np.float32).astype(np.float16),
            "b_kj": np.asarray(inputs["b_kj"], np.float32)[:, None],
            "wc_rbf": wc_rbf,
            "w_down": np.asarray(inputs["W_down"], np.float32).astype(np.float16),
            "w_up2": w_up2,
            "w_res": w_res,
            "b_res": b_res,
        })

    nc = _build_program(E, H, NR, e_core, e_pad, n_pairs, cap_c,
                        NB_BEFORE, NB_AFTER)
    import os as _os
    if _os.environ.get("KSIM"):
        from concourse import bass_interp
        sim = bass_interp.MultiCoreSim(nc, N_CORES)
        for i in range(N_CORES):
            for name, arr in in_maps[i].items():
                sim.cores[i].tensor(name)[:] = arr
        sim.simulate()
        out = np.empty((E, H), np.float32)
        for k in range(N_CORES):
            out[k * e_core:(k + 1) * e_core] = \
                np.asarray(sim.cores[k].tensor("out"))[:, :e_core].T
        return out
    res = run_bass_kernel_spmd(nc, in_maps, list(range(N_CORES)),
                               trace=bool(_os2.environ.get("KTRACE")))
    if res.exec_time_ns is not None:
        print(f"HW exec time: {res.exec_time_ns} ns")

    out = np.empty((E, H), np.float32)
    for k in range(N_CORES):
        out[k * e_core:(k + 1) * e_core] = \
            res.results[k]["out"][:, :e_core].T.astype(np.float32)
    return out



# revision 2
# speedup vs baseline: 1.1391x; 1.1391x over previous
"""DimeNet++ interaction block on 8 Trainium2 NeuronCores (Bass/Tile).

Strategy (v3):
- Sort triplets by idx_ji, shard by 25k-edge ranges per core (no all-reduce:
  each core owns its output edge slice).
- Within a core, triplets grouped into 128-edge windows of idx_ji. Within a
  window, triplets sorted by table row (idx_kj) and packed into NT=11 tiles
  of 128 slots; tile t gathers through a SLIDING 32k-row view of the table
  (static base per t) so int16 gather indices cover the full 200k-row table
  without the old 7-chunk cell padding (1.10x slot padding vs 1.43x).
- Windows processed in PAIRS: window A accumulates in PSUM partitions 0-63,
  window B in 64-127 (one-hot segment-sum matmuls, start/stop chains).
- agg stored as two [64, n_pairs*128] SBUF tiles (lo/hi) so phase C matmuls
  read base-0 operands directly - no DRAM roundtrip.
- Phase C (edge output + residual stack) interleaved into phase B: each
  512-edge block runs as soon as its two window pairs complete, hiding the
  old 645us serial tail under the Q7 gather descriptor generation.
- sbf basis folded on host: se = (sbf@W_sbf1)@W_sbf2 shipped as [T,64] f16.
- x_kj down-projection table [E,64] f16 computed E-sharded, AllGathered.
"""
import sys
import os as _os2
import numpy as np

sys.path.insert(0, "/opt/trn_rl_repo")

N_CORES = 8
WIN_E = 128          # edges per segment-sum window (one PSUM column block)
TILE_T = 128         # triplet slots per matmul tile
NT = 11              # gather tiles per window (sliding-base)
G_PAIRS = 7          # window pairs per group (14 windows per group)
GATHER_SPAN = 32768  # rows addressable per gather call (int16 idx)
BASE_DELTA = 8000    # slack subtracted from nominal sliding bases


def _apply_tile_patches():
    """walrus in this container allows only 1 sync wait per instruction; split
    the TileContext tail drain into a chain of single-wait NOPs. Also register
    the NTFF profile hook so trace=True works (used by test harness only)."""
    import types
    import concourse.tile as tile
    from concourse.vector_clock import ScopedClock

    def _drain_and_barrier_split(self, tick_clock, wait_clock):
        gc = tick_clock.global_clock
        procs = [i for i in range(len(gc)) if gc[i] > 0]
        chunks = [procs[i : i + 1] for i in range(len(procs))]
        for ch in chunks[:-1] if chunks else []:
            nop = self.nc.sync.nop()
            pc = ScopedClock()
            for p in ch:
                pc.require_at_least(None, p, gc[p])
            wait_clock.add_sem_waits(nop.ins, pc)
        drain_inst = self.nc.sync.drain()
        pc = ScopedClock()
        for p in chunks[-1] if chunks else []:
            pc.require_at_least(None, p, gc[p])
        wait_clock.add_sem_waits(drain_inst.ins, pc)
        self.nc.all_engine_barrier()
        assert self.sems is not None
        popped = self.nc._tile_sem_poison_stack.pop()
        assert popped is self._sem_poison
        self.nc.clear_and_free_semaphores(list(self.sems.allocated().values()))
        self.nc.all_engine_barrier()

    tile.TileContext._drain_and_barrier = _drain_and_barrier_split

    if "antenv.axon_hooks" not in sys.modules:
        mod = types.ModuleType("antenv.axon_hooks")
        _state = {"hook": None}
        mod.set_axon_ntff_profile_hook = lambda h: _state.__setitem__("hook", h)
        mod.get_axon_ntff_profile_hook = lambda: _state["hook"]
        sys.modules["antenv.axon_hooks"] = mod
        import antenv

        antenv.axon_hooks = mod
        try:
            from trn_agent_boot.trn_boot import _ntff_profile_via_ctypes

            hook = _ntff_profile_via_ctypes("/opt/axon/libaxon_pjrt.so")
            if hook is not None:
                mod.set_axon_ntff_profile_hook(hook)
        except Exception:
            pass


def _build_program(E, H, NR, e_core, e_pad, n_pairs, bases,
                   NB_BEFORE, NB_AFTER):
    import concourse.bass as bass
    import concourse.bacc as bacc
    import concourse.tile as tile
    from concourse import mybir
    from contextlib import ExitStack

    f16, f32 = mybir.dt.float16, mybir.dt.float32
    AF = mybir.ActivationFunctionType
    OP = mybir.AluOpType

    NB512 = e_pad // 512
    e_tab = N_CORES * e_pad
    n_res = NB_BEFORE * 2 + 1 + NB_AFTER * 2
    n_groups = n_pairs // G_PAIRS
    assert n_pairs % G_PAIRS == 0
    WPG = 2 * G_PAIRS                   # windows per group
    bg = NT * WPG                       # gather blocks per group
    bp = NT * 2                         # one-hot blocks per pair
    n_idx = WPG * TILE_T                # indices per gather call
    nidx16 = n_idx // 16
    n_blocks = NB512                    # phase-C 512-edge blocks

    nc = bacc.Bacc("TRN2", target_bir_lowering=False, debug=False,
                   num_devices=N_CORES)

    def din(name, shape, dt):
        return nc.dram_tensor(name, shape, dt, kind="ExternalInput")

    xT_in = din("xT", [H, e_pad], f16)
    rbfT_in = din("rbfT", [NR, e_pad], f16)
    se_in = din("se", [n_groups, 128, bg * 64], f16)
    gidx_in = din("gidx", [n_groups, NT, 128, nidx16], mybir.dt.int16)
    jil_in = din("jil", [n_pairs, 128, bp, 1], f16)
    iota_in = din("iota", [128, bp, WIN_E], f16)
    w_ji_in = din("w_ji", [H, H], f16)
    b_ji_in = din("b_ji", [H, 1], f32)
    w_kj_in = din("w_kj", [H, H], f16)
    b_kj_in = din("b_kj", [H, 1], f32)
    wc_rbf_in = din("wc_rbf", [NR, H], f16)
    w_down_in = din("w_down", [H, 64], f16)
    w_up_in = din("w_up", [64, H], f16)
    w_res_in = din("w_res", [H, n_res * H], f16)
    b_res_in = din("b_res", [H, n_res], f32)

    out_ext = nc.dram_tensor("out", [H, e_pad], f16, kind="ExternalOutput")

    with tile.TileContext(nc) as tc, ExitStack() as ctx:
        const = ctx.enter_context(tc.tile_pool(name="const", bufs=1))
        persist = ctx.enter_context(tc.tile_pool(name="persist", bufs=1))
        dram = ctx.enter_context(tc.tile_pool(name="dram", bufs=1, space="DRAM"))
        tab_slice = dram.tile([e_pad, 128], f16, tag="tab_slice")
        tab_full = dram.tile([e_tab, 128], f16, tag="tab_full")

        def load_const(ap_in, shape, dt, tag):
            t = const.tile(shape, dt, tag=tag)
            nc.sync.dma_start(t[:], ap_in[:])
            return t

        iota = load_const(iota_in, [128, bp, WIN_E], f16, "c_iota")
        w_ji = load_const(w_ji_in, [H, H], f16, "c_wji")
        b_ji = load_const(b_ji_in, [H, 1], f32, "c_bji")
        w_kj = load_const(w_kj_in, [H, H], f16, "c_wkj")
        b_kj = load_const(b_kj_in, [H, 1], f32, "c_bkj")
        wc_rbf = load_const(wc_rbf_in, [NR, H], f16, "c_wcrbf")
        w_down = load_const(w_down_in, [H, 64], f16, "c_wdown")
        w_up = load_const(w_up_in, [64, H], f16, "c_wup")
        w_res = load_const(w_res_in, [H, n_res * H], f16, "c_wres")
        b_res = load_const(b_res_in, [H, n_res], f32, "c_bres")

        agg_lo = persist.tile([64, n_pairs * WIN_E], f16, tag="agg_lo")
        agg_hi = persist.tile([64, n_pairs * WIN_E], f16, tag="agg_hi")

        # ---------------- phase A: x_kj down-projection table ------------
        with (
            tc.tile_pool(name="a_sb", bufs=3) as a_sb,
            tc.tile_pool(name="a_ps", bufs=2, space="PSUM") as a_ps,
        ):
            for blk in range(NB512):
                sl = slice(blk * 512, (blk + 1) * 512)
                xT = a_sb.tile([H, 512], f16, tag="xT")
                nc.sync.dma_start(xT[:], xT_in[:, sl])
                rbfT = a_sb.tile([NR, 512], f16, tag="rbfT")
                nc.sync.dma_start(rbfT[:], rbfT_in[:, sl])

                ps_kj = a_ps.tile([H, 512], f32, tag="psA")
                nc.tensor.matmul(out=ps_kj[:], lhsT=w_kj[:], rhs=xT[:],
                                 start=True, stop=True)
                t1 = a_sb.tile([H, 512], f16, tag="t1")
                nc.scalar.activation(t1[:], ps_kj[:], AF.Silu, bias=b_kj[:])

                ps_rbf = a_ps.tile([H, 512], f32, tag="psA")
                nc.tensor.matmul(out=ps_rbf[:], lhsT=wc_rbf[:], rhs=rbfT[:],
                                 start=True, stop=True)
                t2 = a_sb.tile([H, 512], f16, tag="t2")
                nc.vector.tensor_tensor(out=t2[:], in0=t1[:], in1=ps_rbf[:],
                                        op=OP.mult)
                td = a_sb.tile([128, 4, 64], f16, tag="td")
                for j in range(4):
                    ps_d = a_ps.tile([128, 64], f32, tag="psD")
                    nc.tensor.matmul(out=ps_d[:],
                                     lhsT=t2[:, j * 128:(j + 1) * 128],
                                     rhs=w_down[:], start=True, stop=True)
                    nc.scalar.activation(td[:, j, :], ps_d[:], AF.Silu)
                nc.sync.dma_start(
                    tab_slice[blk * 512:(blk + 1) * 512, 0:64].rearrange(
                        "(b p) d -> p b d", p=128), td[:])

        # ---------------- AllGather the table ----------------------------
        nc.gpsimd.collective_compute(
            "AllGather", OP.bypass,
            replica_groups=[list(range(N_CORES))],
            ins=[tab_slice.opt()],
            outs=[tab_full.opt()],
        )

        # ---------------- phase B + interleaved phase C -------------------
        with (
            tc.tile_pool(name="b_io", bufs=2) as b_io,
            tc.tile_pool(name="b_gat", bufs=4) as b_gat,
            tc.tile_pool(name="b_m", bufs=2) as b_m,
            tc.tile_pool(name="b_oh", bufs=3) as b_oh,
            tc.tile_pool(name="b_ps", bufs=2, space="PSUM") as b_ps,
            tc.tile_pool(name="c_sb", bufs=3) as c_sb,
            tc.tile_pool(name="c_ps", bufs=2, space="PSUM") as c_ps,
        ):
            def res_layer(h_in, li):
                ps_a = c_ps.tile([H, 512], f32, tag="psU")
                nc.tensor.matmul(out=ps_a[:],
                                 lhsT=w_res[:, li * H:(li + 1) * H],
                                 rhs=h_in[:], start=True, stop=True)
                inner = c_sb.tile([H, 512], f16, tag="inner")
                nc.scalar.activation(inner[:], ps_a[:], AF.Silu,
                                     bias=b_res[:, li:li + 1])
                ps_b = c_ps.tile([H, 512], f32, tag="psU")
                nc.tensor.matmul(out=ps_b[:],
                                 lhsT=w_res[:, (li + 1) * H:(li + 2) * H],
                                 rhs=inner[:], start=True, stop=True)
                s = c_sb.tile([H, 512], f16, tag="s")
                nc.scalar.activation(s[:], ps_b[:], AF.Silu,
                                     bias=b_res[:, li + 1:li + 2])
                h_out = c_sb.tile([H, 512], f16, tag="h")
                nc.vector.tensor_tensor(out=h_out[:], in0=h_in[:],
                                        in1=s[:], op=OP.add)
                return h_out

            def phase_c_block(blk):
                sl = slice(blk * 512, (blk + 1) * 512)
                xT = c_sb.tile([H, 512], f16, tag="xT2")
                nc.sync.dma_start(xT[:], xT_in[:, sl])

                ps_u = c_ps.tile([H, 512], f32, tag="psU")
                for q in range(4):
                    pr = 2 * blk + q // 2
                    src = agg_lo if q % 2 == 0 else agg_hi
                    nc.tensor.matmul(
                        out=ps_u[:, q * 128:(q + 1) * 128],
                        lhsT=w_up[:],
                        rhs=src[:, pr * WIN_E:(pr + 1) * WIN_E],
                        start=True, stop=True)
                xkju = c_sb.tile([H, 512], f16, tag="xkju")
                nc.scalar.activation(xkju[:], ps_u[:], AF.Silu)

                ps_j = c_ps.tile([H, 512], f32, tag="psU")
                nc.tensor.matmul(out=ps_j[:], lhsT=w_ji[:], rhs=xT[:],
                                 start=True, stop=True)
                xji = c_sb.tile([H, 512], f16, tag="xji")
                nc.scalar.activation(xji[:], ps_j[:], AF.Silu, bias=b_ji[:])

                h = c_sb.tile([H, 512], f16, tag="h")
                nc.vector.tensor_tensor(out=h[:], in0=xji[:], in1=xkju[:],
                                        op=OP.add)
                li = 0
                for _ in range(NB_BEFORE):
                    h = res_layer(h, li)
                    li += 2
                ps_l = c_ps.tile([H, 512], f32, tag="psU")
                nc.tensor.matmul(out=ps_l[:],
                                 lhsT=w_res[:, li * H:(li + 1) * H],
                                 rhs=h[:], start=True, stop=True)
                sl_t = c_sb.tile([H, 512], f16, tag="s")
                nc.scalar.activation(sl_t[:], ps_l[:], AF.Silu,
                                     bias=b_res[:, li:li + 1])
                li += 1
                h = c_sb.tile([H, 512], f16, tag="h")
                nc.vector.tensor_tensor(out=h[:], in0=sl_t[:], in1=xT[:],
                                        op=OP.add)
                for r in range(NB_AFTER):
                    h = res_layer(h, li)
                    li += 2
                nc.sync.dma_start(out_ext[:, sl], h[:])

            blocks_done = 0
            for g in range(n_groups):
                se_t = b_io.tile([128, bg, 64], f16, tag="se")
                nc.sync.dma_start(
                    se_t[:].rearrange("p b d -> p (b d)"), se_in[g])
                gi_t = b_io.tile([128, NT, nidx16], mybir.dt.int16,
                                 tag="gi")
                nc.sync.dma_start(
                    gi_t[:], gidx_in[g].rearrange("t p s -> p t s"))

                m = b_m.tile([128, bg, 64], f16, tag="m")
                for t in range(NT):
                    gat = b_gat.tile([128, WPG, 128], f16, tag="gat")
                    nc.gpsimd.dma_gather(
                        gat[:],
                        tab_full[bases[t]:bases[t] + GATHER_SPAN, :],
                        gi_t[:, t, :],
                        n_idx, n_idx, 128, single_packet=False)
                    nc.vector.tensor_tensor(
                        out=m[:, t * WPG:(t + 1) * WPG, :],
                        in0=gat[:, :, 0:64],
                        in1=se_t[:, t * WPG:(t + 1) * WPG, :],
                        op=OP.mult)

                for pig in range(G_PAIRS):
                    p = g * G_PAIRS + pig
                    jl_t = b_oh.tile([128, bp, 1], f16, tag="jl")
                    nc.sync.dma_start(jl_t[:], jil_in[p])
                    oh = b_oh.tile([128, bp, WIN_E], f16, tag="oh")
                    nc.vector.tensor_tensor(
                        out=oh[:], in0=iota[:],
                        in1=jl_t[:].broadcast_to([128, bp, WIN_E]),
                        op=OP.is_equal)
                    ps = b_ps.tile([128, WIN_E], f32, tag="ps")
                    for half in range(2):
                        for t in range(NT):
                            mb_ = t * WPG + 2 * pig + half
                            ob_ = t * 2 + half
                            nc.tensor.matmul(
                                out=ps[64 * half:64 * half + 64, :],
                                lhsT=m[:, mb_, :],
                                rhs=oh[:, ob_, :],
                                start=(t == 0),
                                stop=(t == NT - 1))
                    nc.scalar.activation(
                        agg_lo[:, p * WIN_E:(p + 1) * WIN_E], ps[0:64, :],
                        AF.Copy)
                    nc.scalar.activation(
                        agg_hi[:, p * WIN_E:(p + 1) * WIN_E], ps[64:128, :],
                        AF.Copy)

                new_done = min(n_blocks, (7 * (g + 1)) // 2)
                for blk in range(blocks_done, new_done):
                    phase_c_block(blk)
                blocks_done = new_done
            for blk in range(blocks_done, n_blocks):
                phase_c_block(blk)

    nc.compile()
    return nc


def _host_layout(idx_kj, idx_ji, se_full, E, e_core, e_pad):
    """Sort triplets by ji, shard by core, assign window/tile slots with
    sliding-base gather tiles. Returns per-core layout arrays + bases."""
    T = idx_kj.shape[0]
    n_win = e_pad // WIN_E
    n_pairs = n_win // 2
    assert n_win % 2 == 0 and n_pairs % G_PAIRS == 0
    n_groups = n_pairs // G_PAIRS
    WPG = 2 * G_PAIRS
    bg = NT * WPG
    bp = NT * 2
    n_idx = WPG * TILE_T
    nidx16 = n_idx // 16
    e_tab = N_CORES * e_pad

    bases = np.clip(e_tab * np.arange(NT) // NT - BASE_DELTA,
                    0, e_tab - GATHER_SPAN).astype(np.int64)

    order = np.argsort(idx_ji, kind="stable")
    ji_sorted = idx_ji[order]
    bounds = np.searchsorted(ji_sorted, np.arange(N_CORES + 1) * e_core)
    kj_row_all = (idx_kj // e_core) * e_pad + idx_kj % e_core

    percore = []
    for k in range(N_CORES):
        oj = order[bounds[k]:bounds[k + 1]]
        ji_l = ji_sorted[bounds[k]:bounds[k + 1]] - k * e_core
        rows = kj_row_all[oj]
        # order by (window, table row)
        ord2 = np.lexsort((rows, ji_l // WIN_E))
        oj = oj[ord2]
        ji_l = ji_l[ord2]
        rows = rows[ord2]
        w_arr = ji_l // WIN_E
        wb = np.searchsorted(w_arr, np.arange(n_win + 1))

        tile_of = np.empty(len(oj), np.int16)
        rank_of = np.empty(len(oj), np.int32)
        for w in range(n_win):
            lo, hi = wb[w], wb[w + 1]
            pos = lo
            for t in range(NT):
                if pos >= hi:
                    break
                hi_row = bases[t] + GATHER_SPAN - 1
                n_take = min(128,
                             np.searchsorted(rows[pos:hi], hi_row,
                                             side="right"))
                if n_take > 0:
                    assert rows[pos] >= bases[t], (
                        f"tile assign fail core{k} w{w} t{t}")
                    tile_of[pos:pos + n_take] = t
                    rank_of[pos:pos + n_take] = np.arange(n_take)
                    pos += n_take
            assert pos == hi, f"window overflow core{k} w{w}: {hi-pos} left"

        # slot coordinates
        pair = w_arr // 2
        half = w_arr & 1
        gidx_g = pair // G_PAIRS
        pig = pair % G_PAIRS
        wig = 2 * pig + half
        t_id = tile_of.astype(np.int64)
        # flat slot index: [g, t, wig, rank]
        slots = ((gidx_g * NT + t_id) * WPG + wig) * TILE_T + rank_of
        n_slots = n_groups * NT * WPG * TILE_T

        se_slots = np.zeros((n_slots, 64), np.float16)
        se_slots[slots] = se_full[oj]
        gidx_flat = np.zeros(n_slots, np.int16)
        gidx_flat[slots] = (rows - bases[t_id]).astype(np.int16)
        jil_flat = np.full(n_slots, -1.0, np.float16)
        jil_flat[slots] = (ji_l % WIN_E).astype(np.float16)

        # se: [g, t, wig, part, 64] -> [g, part, (t wig), 64]
        se_r = se_slots.reshape(n_groups, NT, WPG, 128, 64)
        se_arr = np.ascontiguousarray(
            se_r.transpose(0, 3, 1, 2, 4).reshape(n_groups, 128, bg * 64))
        # gidx: per (g, t): flat (wig, rank) -> [16, nidx16] wrap, repl x8
        gi_r = gidx_flat.reshape(n_groups, NT, n_idx)
        gi16 = gi_r.reshape(n_groups, NT, nidx16, 16).transpose(0, 1, 3, 2)
        gi_arr = np.ascontiguousarray(np.tile(gi16, (1, 1, 8, 1)))
        # jil: [g, t, (pig half), part] -> [pair, part, (t half), 1]
        jl_r = jil_flat.reshape(n_groups, NT, G_PAIRS, 2, 128)
        jl_arr = np.ascontiguousarray(
            jl_r.transpose(0, 2, 4, 1, 3).reshape(n_pairs, 128, bp, 1))
        percore.append((se_arr, gi_arr, jl_arr))
    return percore, bases, n_pairs, n_groups, bg, bp


def kernel(**inputs):
    _apply_tile_patches()
    from concourse.bass_utils import run_bass_kernel_spmd

    x = np.asarray(inputs["x"], np.float32)
    rbf = np.asarray(inputs["rbf"], np.float32)
    sbf = np.asarray(inputs["sbf"], np.float32)
    idx_kj = np.asarray(inputs["idx_kj"]).astype(np.int64)
    idx_ji = np.asarray(inputs["idx_ji"]).astype(np.int64)

    E, H = x.shape
    T, NS_NR = sbf.shape
    NR = rbf.shape[1]
    W_res_before = np.asarray(inputs["W_res_before"], np.float32)
    W_res_after = np.asarray(inputs["W_res_after"], np.float32)
    b_res_before = np.asarray(inputs["b_res_before"], np.float32)
    b_res_after = np.asarray(inputs["b_res_after"], np.float32)
    NB_BEFORE = W_res_before.shape[0]
    NB_AFTER = W_res_after.shape[0]

    assert E % N_CORES == 0
    e_core = E // N_CORES
    e_pad = -(-e_core // 512) * 512

    # fold sbf basis: se = (sbf @ W_sbf1) @ W_sbf2  -> [T, 64] f16
    se_full = ((sbf @ np.asarray(inputs["W_sbf1"], np.float32))
               @ np.asarray(inputs["W_sbf2"], np.float32)).astype(np.float16)
    assert se_full.shape[1] == 64

    percore, bases, n_pairs, n_groups, bg, bp = _host_layout(
        idx_kj, idx_ji, se_full, E, e_core, e_pad)

    wc_rbf = (np.asarray(inputs["W_rbf1"], np.float32)
              @ np.asarray(inputs["W_rbf2"], np.float32)).astype(np.float16)
    n_res = NB_BEFORE * 2 + 1 + NB_AFTER * 2
    w_res = np.concatenate([
        W_res_before.reshape(-1, H, H),
        np.asarray(inputs["W_lin"], np.float32)[None],
        W_res_after.reshape(-1, H, H)])
    w_res = np.ascontiguousarray(
        w_res.transpose(1, 0, 2).reshape(H, -1)).astype(np.float16)
    b_res = np.concatenate([
        b_res_before.reshape(-1, H),
        np.asarray(inputs["b_lin"], np.float32)[None],
        b_res_after.reshape(-1, H)])
    b_res = np.ascontiguousarray(b_res.T)  # [H, n_res] f32
    iota = np.broadcast_to(
        np.arange(WIN_E, dtype=np.float16)[None, None, :],
        (128, bp, WIN_E)).copy()

    in_maps = []
    for k in range(N_CORES):
        se_arr, gi_arr, jl_arr = percore[k]
        xT = np.zeros((H, e_pad), np.float16)
        xT[:, :e_core] = x[k * e_core:(k + 1) * e_core].T
        rbfT = np.zeros((NR, e_pad), np.float16)
        rbfT[:, :e_core] = rbf[k * e_core:(k + 1) * e_core].T
        in_maps.append({
            "xT": xT, "rbfT": rbfT, "se": se_arr, "gidx": gi_arr,
            "jil": jl_arr, "iota": iota,
            "w_ji": np.asarray(inputs["W_ji"], np.float32).astype(np.float16),
            "b_ji": np.asarray(inputs["b_ji"], np.float32)[:, None],
            "w_kj": np.asarray(inputs["W_kj"], np.float32).astype(np.float16),
            "b_kj": np.asarray(inputs["b_kj"], np.float32)[:, None],
            "wc_rbf": wc_rbf,
            "w_down": np.asarray(inputs["W_down"], np.float32).astype(np.float16),
            "w_up": np.asarray(inputs["W_up"], np.float32).astype(np.float16),
            "w_res": w_res,
            "b_res": b_res,
        })

    nc = _build_program(E, H, NR, e_core, e_pad, n_pairs, list(bases),
                        NB_BEFORE, NB_AFTER)
    res = run_bass_kernel_spmd(nc, in_maps, list(range(N_CORES)),
                               trace=bool(_os2.environ.get("KTRACE")))
    if res.exec_time_ns is not None:
        print(f"HW exec time: {res.exec_time_ns} ns")

    out = np.empty((E, H), np.float32)
    for k in range(N_CORES):
        out[k * e_core:(k + 1) * e_core] = \
            res.results[k]["out"][:, :e_core].T.astype(np.float32)
    return out


# revision 5
# speedup vs baseline: 1.3482x; 1.1836x over previous
"""DimeNet++ interaction block on 8 Trainium2 NeuronCores (Bass/Tile).

Strategy (v4):
- Sort triplets by idx_ji, shard by 25k-edge ranges per core (no all-reduce:
  each core owns its output edge slice).
- Within a core, triplets grouped into 128-edge windows of idx_ji. Within a
  window, triplets sorted by table row (idx_kj) and packed into NT=11 tiles
  of 128 slots; tile t gathers through a SLIDING 32k-row view of the table
  (static base per t) so int16 gather indices cover the full 200k-row table
  without chunk-cell padding (1.10x slot padding).
- Gather calls merged across GROUP PAIRS (3584 idx/call) to amortize SWDGE
  per-call fixed cost on the Q7 (the end-to-end bottleneck).
- Table AllGather split into 4 quarter collectives pipelined behind phase A;
  tile index t only needs quarters <= its row range, so the first gathers
  start ~150us in instead of ~530us.
- Windows processed in PAIRS: window A accumulates in PSUM partitions 0-63,
  window B in 64-127 (one-hot segment-sum matmuls, start/stop chains).
- agg stored as two [64, n_pairs*128] SBUF tiles (lo/hi) so phase C matmuls
  read base-0 operands directly - no DRAM roundtrip.
- Phase C (edge output + residual stack) interleaved into phase B, with
  ready blocks emitted as interleaved instruction streams (generator
  round-robin) so their 8-hop matmul/silu chains overlap in the in-order
  engine queues.
- sbf basis folded on host: se = (sbf@W_sbf1)@W_sbf2 shipped as [T,64] f16.
"""
import sys
import os as _os2
import numpy as np

sys.path.insert(0, "/opt/trn_rl_repo")

N_CORES = 8
WIN_E = 128          # edges per segment-sum window (one PSUM column block)
TILE_T = 128         # triplet slots per matmul tile
NT = 11              # gather tiles per window (sliding-base)
G_PAIRS = 7          # window pairs per group (14 windows per group)
GATHER_SPAN = 32768  # rows addressable per gather call (int16 idx)
BASE_DELTA = 8000    # slack subtracted from nominal sliding bases
QBLK = (12, 12, 12, 13)  # phase-A 512-blocks per AllGather quarter


def _apply_tile_patches():
    """walrus in this container allows only 1 sync wait per instruction; split
    the TileContext tail drain into a chain of single-wait NOPs. Also register
    the NTFF profile hook so trace=True works (used by test harness only)."""
    import types
    import concourse.tile as tile
    from concourse.vector_clock import ScopedClock

    def _drain_and_barrier_split(self, tick_clock, wait_clock):
        gc = tick_clock.global_clock
        procs = [i for i in range(len(gc)) if gc[i] > 0]
        chunks = [procs[i : i + 1] for i in range(len(procs))]
        for ch in chunks[:-1] if chunks else []:
            nop = self.nc.sync.nop()
            pc = ScopedClock()
            for p in ch:
                pc.require_at_least(None, p, gc[p])
            wait_clock.add_sem_waits(nop.ins, pc)
        drain_inst = self.nc.sync.drain()
        pc = ScopedClock()
        for p in chunks[-1] if chunks else []:
            pc.require_at_least(None, p, gc[p])
        wait_clock.add_sem_waits(drain_inst.ins, pc)
        self.nc.all_engine_barrier()
        assert self.sems is not None
        popped = self.nc._tile_sem_poison_stack.pop()
        assert popped is self._sem_poison
        self.nc.clear_and_free_semaphores(list(self.sems.allocated().values()))
        self.nc.all_engine_barrier()

    tile.TileContext._drain_and_barrier = _drain_and_barrier_split

    if "antenv.axon_hooks" not in sys.modules:
        mod = types.ModuleType("antenv.axon_hooks")
        _state = {"hook": None}
        mod.set_axon_ntff_profile_hook = lambda h: _state.__setitem__("hook", h)
        mod.get_axon_ntff_profile_hook = lambda: _state["hook"]
        sys.modules["antenv.axon_hooks"] = mod
        import antenv

        antenv.axon_hooks = mod
        try:
            from trn_agent_boot.trn_boot import _ntff_profile_via_ctypes

            hook = _ntff_profile_via_ctypes("/opt/axon/libaxon_pjrt.so")
            if hook is not None:
                mod.set_axon_ntff_profile_hook(hook)
        except Exception:
            pass


def _build_program(E, H, NR, e_core, e_pad, n_pairs, bases,
                   NB_BEFORE, NB_AFTER):
    import concourse.bass as bass
    import concourse.bacc as bacc
    import concourse.tile as tile
    from concourse import mybir
    from contextlib import ExitStack

    f16, f32 = mybir.dt.float16, mybir.dt.float32
    AF = mybir.ActivationFunctionType
    OP = mybir.AluOpType

    NB512 = e_pad // 512
    e_tab = N_CORES * e_pad
    n_res = NB_BEFORE * 2 + 1 + NB_AFTER * 2
    n_groups = n_pairs // G_PAIRS
    assert n_pairs % G_PAIRS == 0 and n_groups % 2 == 0
    n_gp = n_groups // 2                # group pairs
    WPG = 2 * G_PAIRS                   # windows per group
    bg = NT * WPG                       # m blocks per group
    bp = NT * 2                         # one-hot blocks per pair
    n_idx = 2 * WPG * TILE_T            # indices per gather call (2 groups)
    nidx16 = n_idx // 16
    n_blocks = NB512
    qrows = [q * 512 for q in QBLK]
    qoff = [0]
    for q in qrows[:-1]:
        qoff.append(qoff[-1] + q)
    qbase = [0]
    for q in qrows[:-1]:
        qbase.append(qbase[-1] + N_CORES * q)

    nc = bacc.Bacc("TRN2", target_bir_lowering=False, debug=False,
                   num_devices=N_CORES)

    def din(name, shape, dt):
        return nc.dram_tensor(name, shape, dt, kind="ExternalInput")

    xT_in = din("xT", [H, e_pad], f16)
    rbfT_in = din("rbfT", [NR, e_pad], f16)
    se_in = din("se", [n_groups, 128, bg * 64], f16)
    gidx_in = din("gidx", [n_gp, NT, 128, nidx16], mybir.dt.int16)
    jil_in = din("jil", [n_pairs, 128, bp, 1], f16)
    iota_in = din("iota", [128, bp, WIN_E], f16)
    w_ji_in = din("w_ji", [H, H], f16)
    b_ji_in = din("b_ji", [H, 1], f32)
    w_kj_in = din("w_kj", [H, H], f16)
    b_kj_in = din("b_kj", [H, 1], f32)
    wc_rbf_in = din("wc_rbf", [NR, H], f16)
    w_down_in = din("w_down", [H, 64], f16)
    w_up_in = din("w_up", [64, H], f16)
    w_res_in = din("w_res", [H, n_res * H], f16)
    b_res_in = din("b_res", [H, n_res], f32)

    out_ext = nc.dram_tensor("out", [H, e_pad], f16, kind="ExternalOutput")

    with tile.TileContext(nc) as tc, ExitStack() as ctx:
        const = ctx.enter_context(tc.tile_pool(name="const", bufs=1))
        persist = ctx.enter_context(tc.tile_pool(name="persist", bufs=1))
        dram = ctx.enter_context(tc.tile_pool(name="dram", bufs=1, space="DRAM"))
        tab_slice = dram.tile([e_pad, 128], f16, tag="tab_slice")
        tab_full = dram.tile([e_tab, 128], f16, tag="tab_full")

        def load_const(ap_in, shape, dt, tag):
            t = const.tile(shape, dt, tag=tag)
            nc.sync.dma_start(t[:], ap_in[:])
            return t

        iota = load_const(iota_in, [128, bp, WIN_E], f16, "c_iota")
        w_ji = load_const(w_ji_in, [H, H], f16, "c_wji")
        b_ji = load_const(b_ji_in, [H, 1], f32, "c_bji")
        w_kj = load_const(w_kj_in, [H, H], f16, "c_wkj")
        b_kj = load_const(b_kj_in, [H, 1], f32, "c_bkj")
        wc_rbf = load_const(wc_rbf_in, [NR, H], f16, "c_wcrbf")
        w_down = load_const(w_down_in, [H, 64], f16, "c_wdown")
        w_up = load_const(w_up_in, [64, H], f16, "c_wup")
        w_res = load_const(w_res_in, [H, n_res * H], f16, "c_wres")
        b_res = load_const(b_res_in, [H, n_res], f32, "c_bres")

        agg_lo = persist.tile([64, n_pairs * WIN_E], f16, tag="agg_lo")
        agg_hi = persist.tile([64, n_pairs * WIN_E], f16, tag="agg_hi")

        # ---------------- phase A + pipelined quarter AllGathers ----------
        with (
            tc.tile_pool(name="a_sb", bufs=3) as a_sb,
            tc.tile_pool(name="a_ps", bufs=2, space="PSUM") as a_ps,
        ):
            qb = []
            acc = 0
            for q in QBLK:
                acc += q
                qb.append(acc - 1)
            qi = 0
            for blk in range(NB512):
                sl = slice(blk * 512, (blk + 1) * 512)
                xT = a_sb.tile([H, 512], f16, tag="xT")
                nc.sync.dma_start(xT[:], xT_in[:, sl])
                rbfT = a_sb.tile([NR, 512], f16, tag="rbfT")
                nc.sync.dma_start(rbfT[:], rbfT_in[:, sl])

                ps_kj = a_ps.tile([H, 512], f32, tag="psA")
                nc.tensor.matmul(out=ps_kj[:], lhsT=w_kj[:], rhs=xT[:],
                                 start=True, stop=True)
                t1 = a_sb.tile([H, 512], f16, tag="t1")
                nc.scalar.activation(t1[:], ps_kj[:], AF.Silu, bias=b_kj[:])

                ps_rbf = a_ps.tile([H, 512], f32, tag="psA")
                nc.tensor.matmul(out=ps_rbf[:], lhsT=wc_rbf[:], rhs=rbfT[:],
                                 start=True, stop=True)
                t2 = a_sb.tile([H, 512], f16, tag="t2")
                nc.vector.tensor_tensor(out=t2[:], in0=t1[:], in1=ps_rbf[:],
                                        op=OP.mult)
                td = a_sb.tile([128, 4, 64], f16, tag="td")
                for j in range(4):
                    ps_d = a_ps.tile([128, 64], f32, tag="psD")
                    nc.tensor.matmul(out=ps_d[:],
                                     lhsT=t2[:, j * 128:(j + 1) * 128],
                                     rhs=w_down[:], start=True, stop=True)
                    nc.scalar.activation(td[:, j, :], ps_d[:], AF.Silu)
                nc.sync.dma_start(
                    tab_slice[blk * 512:(blk + 1) * 512, 0:64].rearrange(
                        "(b p) d -> p b d", p=128), td[:])
                if qi < 4 and blk == qb[qi]:
                    nc.gpsimd.collective_compute(
                        "AllGather", OP.bypass,
                        replica_groups=[list(range(N_CORES))],
                        ins=[tab_slice[qoff[qi]:qoff[qi] + qrows[qi], :].opt()],
                        outs=[tab_full[qbase[qi]:qbase[qi]
                                       + N_CORES * qrows[qi], :].opt()],
                    )
                    qi += 1

        # ---------------- phase B + interleaved phase C -------------------
        with (
            tc.tile_pool(name="b_io", bufs=2) as b_io,
            tc.tile_pool(name="b_gi", bufs=2) as b_gi,
            tc.tile_pool(name="b_gat", bufs=3) as b_gat,
            tc.tile_pool(name="b_m", bufs=2) as b_m,
            tc.tile_pool(name="b_oh", bufs=2) as b_oh,
            tc.tile_pool(name="b_ps", bufs=2, space="PSUM") as b_ps,
            tc.tile_pool(name="c_sb", bufs=4) as c_sb,
            tc.tile_pool(name="c_ps", bufs=4, space="PSUM") as c_ps,
        ):
            def phase_c_gen(blk):
                sl = slice(blk * 512, (blk + 1) * 512)
                xT = c_sb.tile([H, 512], f16, tag="xT2")
                nc.sync.dma_start(xT[:], xT_in[:, sl])
                ps_u = c_ps.tile([H, 512], f32, tag="psU")
                for q in range(4):
                    pr = 2 * blk + q // 2
                    src = agg_lo if q % 2 == 0 else agg_hi
                    nc.tensor.matmul(
                        out=ps_u[:, q * 128:(q + 1) * 128],
                        lhsT=w_up[:],
                        rhs=src[:, pr * WIN_E:(pr + 1) * WIN_E],
                        start=True, stop=True)
                yield
                xkju = c_sb.tile([H, 512], f16, tag="xkju")
                nc.scalar.activation(xkju[:], ps_u[:], AF.Silu)
                ps_j = c_ps.tile([H, 512], f32, tag="psU")
                nc.tensor.matmul(out=ps_j[:], lhsT=w_ji[:], rhs=xT[:],
                                 start=True, stop=True)
                yield
                xji = c_sb.tile([H, 512], f16, tag="xji")
                nc.scalar.activation(xji[:], ps_j[:], AF.Silu, bias=b_ji[:])
                h = c_sb.tile([H, 512], f16, tag="h")
                nc.vector.tensor_tensor(out=h[:], in0=xji[:], in1=xkju[:],
                                        op=OP.add)
                yield
                li = 0
                for _ in range(NB_BEFORE):
                    ps_a = c_ps.tile([H, 512], f32, tag="psU")
                    nc.tensor.matmul(out=ps_a[:],
                                     lhsT=w_res[:, li * H:(li + 1) * H],
                                     rhs=h[:], start=True, stop=True)
                    yield
                    inner = c_sb.tile([H, 512], f16, tag="inner")
                    nc.scalar.activation(inner[:], ps_a[:], AF.Silu,
                                         bias=b_res[:, li:li + 1])
                    ps_b = c_ps.tile([H, 512], f32, tag="psU")
                    nc.tensor.matmul(out=ps_b[:],
                                     lhsT=w_res[:, (li + 1) * H:(li + 2) * H],
                                     rhs=inner[:], start=True, stop=True)
                    yield
                    s = c_sb.tile([H, 512], f16, tag="s")
                    nc.scalar.activation(s[:], ps_b[:], AF.Silu,
                                         bias=b_res[:, li + 1:li + 2])
                    h2 = c_sb.tile([H, 512], f16, tag="h")
                    nc.vector.tensor_tensor(out=h2[:], in0=h[:], in1=s[:],
                                            op=OP.add)
                    h = h2
                    li += 2
                    yield
                ps_l = c_ps.tile([H, 512], f32, tag="psU")
                nc.tensor.matmul(out=ps_l[:],
                                 lhsT=w_res[:, li * H:(li + 1) * H],
                                 rhs=h[:], start=True, stop=True)
                yield
                sl_t = c_sb.tile([H, 512], f16, tag="s")
                nc.scalar.activation(sl_t[:], ps_l[:], AF.Silu,
                                     bias=b_res[:, li:li + 1])
                li += 1
                h = c_sb.tile([H, 512], f16, tag="h")
                nc.vector.tensor_tensor(out=h[:], in0=sl_t[:], in1=xT[:],
                                        op=OP.add)
                yield
                for r in range(NB_AFTER):
                    ps_a = c_ps.tile([H, 512], f32, tag="psU")
                    nc.tensor.matmul(out=ps_a[:],
                                     lhsT=w_res[:, li * H:(li + 1) * H],
                                     rhs=h[:], start=True, stop=True)
                    yield
                    inner = c_sb.tile([H, 512], f16, tag="inner")
                    nc.scalar.activation(inner[:], ps_a[:], AF.Silu,
                                         bias=b_res[:, li:li + 1])
                    ps_b = c_ps.tile([H, 512], f32, tag="psU")
                    nc.tensor.matmul(out=ps_b[:],
                                     lhsT=w_res[:, (li + 1) * H:(li + 2) * H],
                                     rhs=inner[:], start=True, stop=True)
                    yield
                    s = c_sb.tile([H, 512], f16, tag="s")
                    nc.scalar.activation(s[:], ps_b[:], AF.Silu,
                                         bias=b_res[:, li + 1:li + 2])
                    h2 = c_sb.tile([H, 512], f16, tag="h")
                    nc.vector.tensor_tensor(out=h2[:], in0=h[:], in1=s[:],
                                            op=OP.add)
                    h = h2
                    li += 2
                    yield
                nc.sync.dma_start(out_ext[:, sl], h[:])

            def drain_interleaved(gens, K=3):
                # chunk width must stay <= every per-wave pool bufs count,
                # else the in-order engine queues can deadlock on allocation
                for i in range(0, len(gens), K):
                    chunk = list(gens[i:i + K])
                    while chunk:
                        nxt = []
                        for gi_ in chunk:
                            try:
                                next(gi_)
                                nxt.append(gi_)
                            except StopIteration:
                                pass
                        chunk = nxt

            blocks_done = 0
            for gp in range(n_gp):
                gi_t = b_gi.tile([128, NT, nidx16], mybir.dt.int16, tag="gi")
                nc.sync.dma_start(
                    gi_t[:], gidx_in[gp].rearrange("t p s -> p t s"))
                se_g = []
                m_g = []
                for g2 in range(2):
                    g = 2 * gp + g2
                    se_t = b_io.tile([128, bg, 64], f16, tag="se")
                    nc.sync.dma_start(
                        se_t[:].rearrange("p b d -> p (b d)"), se_in[g])
                    se_g.append(se_t)
                    m_t = b_m.tile([128, bg, 64], f16, tag="m")
                    m_g.append(m_t)

                for t in range(NT):
                    gat = b_gat.tile([128, 2 * WPG, 128], f16, tag="gat")
                    nc.gpsimd.dma_gather(
                        gat[:],
                        tab_full[bases[t]:bases[t] + GATHER_SPAN, :],
                        gi_t[:, t, :],
                        n_idx, n_idx, 128, single_packet=False)
                    for g2 in range(2):
                        nc.vector.tensor_tensor(
                            out=m_g[g2][:, t * WPG:(t + 1) * WPG, :],
                            in0=gat[:, g2 * WPG:(g2 + 1) * WPG, 0:64],
                            in1=se_g[g2][:, t * WPG:(t + 1) * WPG, :],
                            op=OP.mult)

                for g2 in range(2):
                    g = 2 * gp + g2
                    m = m_g[g2]
                    for pig in range(G_PAIRS):
                        p = g * G_PAIRS + pig
                        jl_t = b_oh.tile([128, bp, 1], f16, tag="jl")
                        nc.sync.dma_start(jl_t[:], jil_in[p])
                        oh = b_oh.tile([128, bp, WIN_E], f16, tag="oh")
                        nc.vector.tensor_tensor(
                            out=oh[:], in0=iota[:],
                            in1=jl_t[:].broadcast_to([128, bp, WIN_E]),
                            op=OP.is_equal)
                        ps = b_ps.tile([128, WIN_E], f32, tag="ps")
                        for half in range(2):
                            for t in range(NT):
                                mb_ = t * WPG + 2 * pig + half
                                ob_ = t * 2 + half
                                nc.tensor.matmul(
                                    out=ps[64 * half:64 * half + 64, :],
                                    lhsT=m[:, mb_, :],
                                    rhs=oh[:, ob_, :],
                                    start=(t == 0),
                                    stop=(t == NT - 1))
                        nc.scalar.activation(
                            agg_lo[:, p * WIN_E:(p + 1) * WIN_E],
                            ps[0:64, :], AF.Copy)
                        nc.scalar.activation(
                            agg_hi[:, p * WIN_E:(p + 1) * WIN_E],
                            ps[64:128, :], AF.Copy)

                new_done = min(n_blocks, (G_PAIRS * 2 * (gp + 1)) // 2)
                drain_interleaved(
                    [phase_c_gen(b) for b in range(blocks_done, new_done)])
                blocks_done = new_done
            drain_interleaved(
                [phase_c_gen(b) for b in range(blocks_done, n_blocks)])

    nc.compile()
    return nc


def _host_layout(idx_kj, idx_ji, se_full, E, e_core, e_pad):
    """Sort triplets by ji, shard by core, assign window/tile slots with
    sliding-base gather tiles over the quarter-major table row space."""
    n_win = e_pad // WIN_E
    n_pairs = n_win // 2
    assert n_win % 2 == 0 and n_pairs % G_PAIRS == 0
    n_groups = n_pairs // G_PAIRS
    assert n_groups % 2 == 0
    n_gp = n_groups // 2
    WPG = 2 * G_PAIRS
    bg = NT * WPG
    bp = NT * 2
    n_idx = 2 * WPG * TILE_T
    nidx16 = n_idx // 16
    e_tab = N_CORES * e_pad

    qrows = np.array([q * 512 for q in QBLK])
    qoff = np.concatenate(([0], np.cumsum(qrows)[:-1]))
    qbase = np.concatenate(([0], np.cumsum(N_CORES * qrows)[:-1]))
    assert qrows.sum() == e_pad

    bases = np.clip(e_tab * np.arange(NT) // NT - BASE_DELTA,
                    0, e_tab - GATHER_SPAN).astype(np.int64)

    order = np.argsort(idx_ji, kind="stable")
    ji_sorted = idx_ji[order]
    bounds = np.searchsorted(ji_sorted, np.arange(N_CORES + 1) * e_core)
    k_arr = idx_kj // e_core
    r_arr = idx_kj % e_core
    q_arr = np.searchsorted(qoff, r_arr, side="right") - 1
    kj_row_all = qbase[q_arr] + k_arr * qrows[q_arr] + (r_arr - qoff[q_arr])

    percore = []
    for k in range(N_CORES):
        oj = order[bounds[k]:bounds[k + 1]]
        ji_l = ji_sorted[bounds[k]:bounds[k + 1]] - k * e_core
        rows = kj_row_all[oj]
        ord2 = np.lexsort((rows, ji_l // WIN_E))
        oj = oj[ord2]
        ji_l = ji_l[ord2]
        rows = rows[ord2]
        w_arr = ji_l // WIN_E
        wb = np.searchsorted(w_arr, np.arange(n_win + 1))

        tile_of = np.empty(len(oj), np.int16)
        rank_of = np.empty(len(oj), np.int32)
        for w in range(n_win):
            lo, hi = wb[w], wb[w + 1]
            pos = lo
            for t in range(NT):
                if pos >= hi:
                    break
                hi_row = bases[t] + GATHER_SPAN - 1
                n_take = min(128,
                             np.searchsorted(rows[pos:hi], hi_row,
                                             side="right"))
                if n_take > 0:
                    assert rows[pos] >= bases[t], (
                        f"tile assign fail core{k} w{w} t{t}")
                    tile_of[pos:pos + n_take] = t
                    rank_of[pos:pos + n_take] = np.arange(n_take)
                    pos += n_take
            assert pos == hi, f"window overflow core{k} w{w}: {hi-pos} left"

        pair = w_arr // 2
        half = w_arr & 1
        g_of = pair // G_PAIRS
        pig = pair % G_PAIRS
        wig = 2 * pig + half
        t_id = tile_of.astype(np.int64)
        slots = ((g_of * NT + t_id) * WPG + wig) * TILE_T + rank_of
        n_slots = n_groups * NT * WPG * TILE_T

        se_slots = np.zeros((n_slots, 64), np.float16)
        se_slots[slots] = se_full[oj]
        gidx_flat = np.zeros(n_slots, np.int16)
        gidx_flat[slots] = (rows - bases[t_id]).astype(np.int16)
        jil_flat = np.full(n_slots, -1.0, np.float16)
        jil_flat[slots] = (ji_l % WIN_E).astype(np.float16)

        # se: [g, t, wig, part, 64] -> [g, part, (t wig), 64]
        se_r = se_slots.reshape(n_groups, NT, WPG, 128, 64)
        se_arr = np.ascontiguousarray(
            se_r.transpose(0, 3, 1, 2, 4).reshape(n_groups, 128, bg * 64))
        # gidx: [g, t, wig*128] -> [gp, t, (g2, wig, rank)] wrap16, repl x8
        gi_r = gidx_flat.reshape(n_gp, 2, NT, WPG * TILE_T)
        gi_r = gi_r.transpose(0, 2, 1, 3).reshape(n_gp, NT, n_idx)
        gi16 = gi_r.reshape(n_gp, NT, nidx16, 16).transpose(0, 1, 3, 2)
        gi_arr = np.ascontiguousarray(np.tile(gi16, (1, 1, 8, 1)))
        # jil: [g, t, (pig half), part] -> [pair, part, (t half), 1]
        jl_r = jil_flat.reshape(n_groups, NT, G_PAIRS, 2, 128)
        jl_arr = np.ascontiguousarray(
            jl_r.transpose(0, 2, 4, 1, 3).reshape(n_pairs, 128, bp, 1))
        percore.append((se_arr, gi_arr, jl_arr))
    return percore, bases, n_pairs, n_groups, bg, bp


def kernel(**inputs):
    _apply_tile_patches()
    from concourse.bass_utils import run_bass_kernel_spmd

    x = np.asarray(inputs["x"], np.float32)
    rbf = np.asarray(inputs["rbf"], np.float32)
    sbf = np.asarray(inputs["sbf"], np.float32)
    idx_kj = np.asarray(inputs["idx_kj"]).astype(np.int64)
    idx_ji = np.asarray(inputs["idx_ji"]).astype(np.int64)

    E, H = x.shape
    T, NS_NR = sbf.shape
    NR = rbf.shape[1]
    W_res_before = np.asarray(inputs["W_res_before"], np.float32)
    W_res_after = np.asarray(inputs["W_res_after"], np.float32)
    b_res_before = np.asarray(inputs["b_res_before"], np.float32)
    b_res_after = np.asarray(inputs["b_res_after"], np.float32)
    NB_BEFORE = W_res_before.shape[0]
    NB_AFTER = W_res_after.shape[0]

    assert E % N_CORES == 0
    e_core = E // N_CORES
    e_pad = -(-e_core // 512) * 512

    se_full = ((sbf @ np.asarray(inputs["W_sbf1"], np.float32))
               @ np.asarray(inputs["W_sbf2"], np.float32)).astype(np.float16)
    assert se_full.shape[1] == 64

    percore, bases, n_pairs, n_groups, bg, bp = _host_layout(
        idx_kj, idx_ji, se_full, E, e_core, e_pad)

    wc_rbf = (np.asarray(inputs["W_rbf1"], np.float32)
              @ np.asarray(inputs["W_rbf2"], np.float32)).astype(np.float16)
    n_res = NB_BEFORE * 2 + 1 + NB_AFTER * 2
    w_res = np.concatenate([
        W_res_before.reshape(-1, H, H),
        np.asarray(inputs["W_lin"], np.float32)[None],
        W_res_after.reshape(-1, H, H)])
    w_res = np.ascontiguousarray(
        w_res.transpose(1, 0, 2).reshape(H, -1)).astype(np.float16)
    b_res = np.concatenate([
        b_res_before.reshape(-1, H),
        np.asarray(inputs["b_lin"], np.float32)[None],
        b_res_after.reshape(-1, H)])
    b_res = np.ascontiguousarray(b_res.T)  # [H, n_res] f32
    iota = np.broadcast_to(
        np.arange(WIN_E, dtype=np.float16)[None, None, :],
        (128, bp, WIN_E)).copy()

    in_maps = []
    for k in range(N_CORES):
        se_arr, gi_arr, jl_arr = percore[k]
        xT = np.zeros((H, e_pad), np.float16)
        xT[:, :e_core] = x[k * e_core:(k + 1) * e_core].T
        rbfT = np.zeros((NR, e_pad), np.float16)
        rbfT[:, :e_core] = rbf[k * e_core:(k + 1) * e_core].T
        in_maps.append({
            "xT": xT, "rbfT": rbfT, "se": se_arr, "gidx": gi_arr,
            "jil": jl_arr, "iota": iota,
            "w_ji": np.asarray(inputs["W_ji"], np.float32).astype(np.float16),
            "b_ji": np.asarray(inputs["b_ji"], np.float32)[:, None],
            "w_kj": np.asarray(inputs["W_kj"], np.float32).astype(np.float16),
            "b_kj": np.asarray(inputs["b_kj"], np.float32)[:, None],
            "wc_rbf": wc_rbf,
            "w_down": np.asarray(inputs["W_down"], np.float32).astype(np.float16),
            "w_up": np.asarray(inputs["W_up"], np.float32).astype(np.float16),
            "w_res": w_res,
            "b_res": b_res,
        })

    nc = _build_program(E, H, NR, e_core, e_pad, n_pairs, list(bases),
                        NB_BEFORE, NB_AFTER)
    res = run_bass_kernel_spmd(nc, in_maps, list(range(N_CORES)),
                               trace=bool(_os2.environ.get("KTRACE")))
    if res.exec_time_ns is not None:
        print(f"HW exec time: {res.exec_time_ns} ns")

    out = np.empty((E, H), np.float32)
    for k in range(N_CORES):
        out[k * e_core:(k + 1) * e_core] = \
            res.results[k]["out"][:, :e_core].T.astype(np.float32)
    return out
